# revision 67
# baseline (speedup 1.0000x reference)
"""Trainium2 Bass kernel for nn_AxialBlock (axial attention, branches W/H/T).

Self-contained: accepts FULL inputs as in reference.setup_inputs(), shards
across 8 NeuronCores as (batch x head-half), runs one SPMD Bass program,
gathers on host.

Hardcoded problem shape: x (4, 512, 16, 32, 32) f32, C=512, 8 heads, d=64.

Per-core layout: activations channel-major [C, tokens]. The work is a single
stream of 48 uniform 1024-token "units": 16 t-planes x (W branch, H branch)
then 16 h-row-pairs (T branch, combined with the W+H partial from a DRAM
scratch). Branch token orders (W: (h,w) natural; H: (w,h); T: (r,w,t)) come
from DVE reorder copies pipelined one unit ahead.

The scheduling is the point: for consecutive units, the loop emits
  [unit i-1's attention/out-proj items interleaved inside unit i's
   projection acc-groups]
so the PE sees an unbroken dense matmul stream (stays at its fast p-state)
while each unit's softmax chain (Act exp -> DVE reduce/reciprocal ->
normalize muls split Act/DVE -> DVE transpose) drains underneath the next
unit's projections. Attention per 128-token group (4 sequences x 32 tokens)
is tile_position-packed; the two concurrently-streaming PE row groups write
separate PSUM banks (h strides one bank in the score tile) -- sharing a bank
between row groups faults on hardware.
"""

import numpy as np

import concourse.bass as bass
import concourse.mybir as mybir
from concourse import bacc, tile
from concourse.bass_utils import run_bass_kernel_spmd

F32 = mybir.dt.float32
F32R = mybir.dt.float32r
BF16 = mybir.dt.bfloat16
F8 = mybir.dt.float8e4
AF = mybir.ActivationFunctionType
ALU = mybir.AluOpType

B, C, T, H, W = 4, 512, 16, 32, 32
NH, D = 8, 64
HH = 4  # heads per core (head-half)
CH = HH * D  # 256 channels per core
NEG = -30000.0


def build_nc():
    nc = bacc.Bacc("TRN2", target_bir_lowering=False, debug=False, num_devices=8)

    x_in = nc.dram_tensor("x_in", [C, T, H, W], F32, kind="ExternalInput")
    # qkv weights (q UNSCALED -- the 1/sqrt(d) folds into exp's scale); the
    # W branch's q/k additionally pre-cast to fp8e4m3 for DoubleRow
    wqkv = {
        ax: nc.dram_tensor(f"wqkv_{ax}", [C, 3 * CH], F32, kind="ExternalInput")
        for ax in ("w", "h", "t")
    }
    wqk8_w = nc.dram_tensor("wqk8_w", [C, 2 * CH], F8, kind="ExternalInput")
    fc = {
        ax: nc.dram_tensor(f"fc_{ax}", [CH, C], F32, kind="ExternalInput")
        for ax in ("w", "h", "t")
    }
    # One output per branch, each in that branch's natural token order
    # (contiguous drains + contiguous DMA); the host transposes and sums.
    y_w = nc.dram_tensor("y_w", [C, T, H * W], BF16, kind="ExternalOutput")
    y_h = nc.dram_tensor("y_h", [C, T, W * H], BF16, kind="ExternalOutput")
    y_t = nc.dram_tensor("y_t", [C, H, W * T], BF16, kind="ExternalOutput")

    # T-branch pair mask over (h, c, g2, m): within a 32-token col strip the
    # two 16-token sequences must not attend to each other.
    mrows = np.arange(128) % 32
    mcols = np.arange(32)
    m2 = np.where((mrows[:, None] // 16) == (mcols[None, :] // 16), 0.0, NEG)
    mask_np = np.broadcast_to(
        m2[:, None, None, None, :], (128, 2, 2, 4, 32)
    ).astype(np.float32)
    mask_dram = nc.inline_tensor(np.ascontiguousarray(mask_np), name="tmask")

    with tile.TileContext(nc) as tc:
        with (
            tc.tile_pool(name="consts", bufs=1) as consts,
            tc.tile_pool(name="xtp", bufs=2) as xtp,
            tc.tile_pool(name="qkv", bufs=2) as qkvp,
            tc.tile_pool(name="att", bufs=4) as attp,
            tc.tile_pool(name="yp", bufs=2) as yp,
            tc.tile_pool(name="ps", bufs=1, space="PSUM") as ps,
        ):
            w_t = {}
            fc_t = {}
            for ax in ("w", "h", "t"):
                w_t[ax] = consts.tile([128, 4, 3 * CH], BF16, name=f"w_{ax}")
                nc.gpsimd.dma_start(
                    out=w_t[ax],
                    in_=wqkv[ax].rearrange("(kc kp) m -> kp kc m", kp=128),
                )
                fc_t[ax] = consts.tile([128, 2, C], F32R, name=f"fc_{ax}")
                nc.sync.dma_start(
                    out=fc_t[ax],
                    in_=fc[ax].rearrange("(kc kp) m -> kp kc m", kp=128)
                    .bitcast(F32R),
                )
            # DoubleRow layout for the W branch: c = ic2*256 + ko*128 + p
            w8_t = {}
            w8_t["w"] = consts.tile([128, 2, 2, 2 * CH], F8, name="w8_w")
            nc.sync.dma_start(
                out=w8_t["w"],
                in_=wqk8_w.rearrange(
                    "(ic ko kp) m -> kp ic ko m", kp=128, ko=2),
            )
            mask_t = consts.tile([128, 2, 2, 4, 32], F32, name="mask_t")
            nc.sync.dma_start(out=mask_t, in_=mask_dram[:, :, :, :, :])

            def qk_mms(xv, ax, ntok, qt, kt, fill):
                """q/k projection. For the W branch (whose branch token order
                IS the natural plane order) the matmuls run in fp8 DoubleRow:
                2 matmuls per acc group over 256 contraction channels each,
                with xv(ic2, tt) -> [128, 2, 512] fp8 rhs. H/T branches run
                bf16 (4 matmuls) on their reordered x tiles -- an fp8 copy in
                branch order would cost more to produce than DoubleRow saves.
                Drains are contiguous either way. fill() runs one interleaved
                attention item per acc-group."""
                fp8 = ax == "w"
                for tt in range(ntok // 512):
                    for mc in range(4):  # q0 q1 k0 k1
                        dst = qt if mc < 2 else kt
                        oc = mc % 2
                        acc = ps.tile([128, 512], F32, name="acc", tag="big",
                                      bufs=2)
                        if fp8:
                            for ic2 in range(2):
                                nc.tensor.matmul(
                                    acc,
                                    w8_t[ax][:, ic2, :,
                                             mc * 128 : (mc + 1) * 128],
                                    xv(ic2, tt),
                                    start=(ic2 == 0),
                                    stop=(ic2 == 1),
                                    perf_mode=mybir.MatmulPerfMode.DoubleRow,
                                )
                        else:
                            for ic in range(4):
                                nc.tensor.matmul(
                                    acc,
                                    w_t[ax][:, ic,
                                            mc * 128 : (mc + 1) * 128],
                                    xv(ic, tt),
                                    start=(ic == 0),
                                    stop=(ic == 3),
                                )
                        nc.scalar.copy(
                            out=dst[:, oc, tt * 512 : (tt + 1) * 512],
                            in_=acc,
                        )
                        fill()

            def v_mms(xg, ax, ntok, vr, fill):
                """v projection, token-major. xg: callable(ic, g) -> lhsT AP
                [128, 128] = tokens g*128..+128 in branch order. Two fill
                points per group (one mid-accumulation) so the filler list
                drains before the unit boundary -- trailing drains otherwise
                stall the next unit's first acc groups on the PSUM WAR."""
                for gp in range(0, ntok // 128, 2):
                    # two groups share one acc tile (1 bank): full-128-row
                    # stationaries never co-stream, so same-bank is safe;
                    # ONE drain covers both groups
                    acc2 = ps.tile([128, 2, 256], F32, name="acc2",
                                   tag="big", bufs=2)
                    for gj in range(2):
                        for ic in range(4):
                            nc.tensor.matmul(
                                acc2[:, gj, :],
                                xg(ic, gp + gj),
                                w_t[ax][:, ic, 512:768],
                                start=(ic == 0),
                                stop=(ic == 3),
                            )
                            if ic == 1:
                                fill()
                        fill()
                    if gp % 4 == 0:
                        nc.vector.tensor_copy(
                            out=vr[:, gp : gp + 2, :], in_=acc2)
                    else:
                        nc.scalar.copy(out=vr[:, gp : gp + 2, :], in_=acc2)

            def score_mms(g, sct, qt, kt):
                # sct [128, 2(h), 2(c), 8(g), 32(m)]: h strides one full PSUM
                # bank, so the two concurrently-streaming tile_position row
                # groups never share a bank (same constraint the per-h tiles
                # of the baseline satisfied).
                for s in range(4):
                    q32 = slice(g * 128 + s * 32, g * 128 + (s + 1) * 32)
                    for c in range(2):
                        for h in range(2):
                            nc.tensor.matmul(
                                sct[s * 32 : (s + 1) * 32, h, c, g, :],
                                qt[h * 64 : (h + 1) * 64, c, q32],
                                kt[h * 64 : (h + 1) * 64, c, q32],
                                start=True,
                                stop=True,
                                tile_position=(h * 64, s * 32),
                                skip_group_check=True,
                            )
            def softmax_quad(g, sct, masked):
                """Softmax for groups g..g+3 batched: one mask add, one exp,
                one reduce, one reciprocal, ONE broadcast normalize mul (on
                the otherwise-idle GpSimd engine; all operands SBUF), and ONE
                32x32-block stream transpose for the whole quad. The (h, c)
                layout is kept end-to-end; av_mms indexes attT accordingly.
                Returns the quad's attT tile."""
                scv = sct[:, :, :, g : g + 4, :].rearrange(
                    "p h c g m -> p h c (g m)"
                )
                if masked:
                    nc.vector.tensor_tensor(
                        out=scv, in0=scv,
                        in1=mask_t.rearrange("p h c g m -> p h c (g m)"),
                        op=ALU.add,
                    )
                # e4 memory layout (h, c, g4, m) == the exp input order
                e4 = attp.tile([128, 2, 2, 4, 32], BF16, name="e4",
                               tag="e2", bufs=4)
                # scores are unscaled (fp8 weights can't carry the tiny
                # pre-scaled q); 1/sqrt(d) folds into the exp input scale
                nc.scalar.activation(
                    out=e4.rearrange("p h c g m -> p h c (g m)"),
                    in_=scv,
                    func=AF.Exp,
                    scale=0.125,
                )
                rs = attp.tile([128, 16], F32, name="rs", tag="rs", bufs=4)
                nc.vector.tensor_reduce(
                    out=rs,
                    in_=e4.rearrange("p h c g m -> p (h c g) m"),
                    axis=mybir.AxisListType.X,
                    op=ALU.add,
                )
                rv = attp.tile([128, 16], F32, name="rv", tag="rv", bufs=4)
                nc.vector.reciprocal(out=rv, in_=rs)
                attn = attp.tile([128, 2, 2, 4, 32], BF16, name="attn",
                                 tag="attn", bufs=4)
                nc.gpsimd.tensor_tensor(
                    out=attn.rearrange("p h c g m -> p (h c g) m"),
                    in0=e4.rearrange("p h c g m -> p (h c g) m"),
                    in1=rv[:, :, None].to_broadcast([128, 16, 32]),
                    op=ALU.mult,
                )
                attT = attp.tile([128, 2, 2, 4, 32], BF16, name="attT",
                                 tag="attT", bufs=4)
                nc.vector.transpose(
                    out=attT.rearrange("p h c g m -> p (h c g m)"),
                    in_=attn.rearrange("p h c g m -> p (h c g m)"),
                )
                return attT

            def av_mms(g, vr, attT, gj, ot):
                # avt [128, 4(s), 512]: every s row group gets its own PSUM
                # bank (strictly more separation than the old av0/av1
                # alternation), then ONE strided DVE drain for the group.
                avt = ps.tile([128, 4, 512], F32, name="av", tag="av",
                              bufs=1)
                for s in range(4):
                    for c in range(2):
                        for h in range(2):
                            nc.tensor.matmul(
                                avt[h * 64 : (h + 1) * 64, s,
                                    c * 32 : (c + 1) * 32],
                                vr[s * 32 : (s + 1) * 32, g,
                                   (2 * c + h) * 64 : (2 * c + h + 1) * 64],
                                attT[s * 32 : (s + 1) * 32, h, c, gj, :],
                                start=True,
                                stop=True,
                                tile_position=(s * 32, h * 64),
                                skip_group_check=True,
                            )
                # split the drain across both engines (halves run in
                # parallel and the bufs=1 WAR clears sooner)
                dst = ot[:, :, g * 128 : (g + 1) * 128].rearrange(
                    "p c (s m) -> p c s m", m=32
                )
                src = avt[:, :, 0:64].rearrange(
                    "p s (c m) -> p c s m", m=32
                )
                nc.vector.tensor_copy(out=dst[:, :, 0:2, :],
                                      in_=src[:, :, 0:2, :])
                nc.scalar.copy(out=dst[:, :, 2:4, :], in_=src[:, :, 2:4, :])

            def op_item(ax, ot, tt, oc, write_fn):
                yps = ps.tile([128, 512], F32, name="yps", tag="big",
                              bufs=2)
                for ic in range(2):
                    nc.tensor.matmul(
                        yps,
                        fc_t[ax][:, ic, oc * 128 : (oc + 1) * 128],
                        ot[:, ic, tt * 512 : (tt + 1) * 512],
                        start=(ic == 0),
                        stop=(ic == 1),
                    )
                write_fn(oc, tt, yps)

            def branch_front(ax, ntok, xv, xg, fillers):
                """Projections for one branch, with the attention items of
                the unit two slots back interleaved between acc-groups so the
                PE never sees a sparse stretch."""
                ng = ntok // 128
                qt = qkvp.tile([128, 2, 1024], BF16, name="qt", tag="qt",
                               bufs=3)[:, :, :ntok]
                kt = qkvp.tile([128, 2, 1024], BF16, name="kt", tag="kt",
                               bufs=3)[:, :, :ntok]
                vr = qkvp.tile([128, 8, 256], BF16, name="vr", tag="vr",
                               bufs=3)[:, :ng, :]
                ot = qkvp.tile([128, 2, 1024], F32R, name="ot", tag="ot",
                               bufs=2)[:, :, :ntok]
                it = iter(fillers)
                state = {"skip": 0}

                def fill():
                    if state["skip"] > 0:  # let qt/kt copies get ahead
                        state["skip"] -= 1
                        return
                    f = next(it, None)
                    if f is not None:
                        f()

                qk_mms(xv, ax, ntok, qt, kt, fill)
                v_mms(xg, ax, ntok, vr, fill)
                for f in it:
                    f()
                return (ax, ntok, qt, kt, vr, ot)

            def attn_fillers(st, masked, write_fn):
                """Interleavable attention + out-projection items for a unit:
                score packs, AV packs, and (tt, oc) out-proj chunks ordered so
                each item's dependencies were issued several items earlier."""
                ax, ntok, qt, kt, vr, ot = st
                ng = ntok // 128
                sct = ps.tile([128, 2, 2, 8, 32], F32, name="sc", tag="sc",
                              bufs=1)
                attTs = [None] * (ng // 4)
                items = []

                def sc_item(gq):
                    for g in range(gq, gq + 4):
                        score_mms(g, sct, qt, kt)
                    attTs[gq // 4] = softmax_quad(gq, sct, masked)

                def av_item(g):
                    av_mms(g, vr, attTs[g // 4], g % 4, ot)

                for gq in range(0, ng, 4):
                    items.append(lambda gq=gq: sc_item(gq))
                for g in range(ng // 2):
                    items.append(lambda g=g: av_item(g))
                for k in range(ng // 2):
                    items.append(lambda g=ng // 2 + k: av_item(g))
                    items.append(
                        lambda oc=k: op_item(ax, ot, 0, oc, write_fn))
                for oc in range(4):
                    items.append(lambda oc=oc: op_item(ax, ot, 1, oc,
                                                       write_fn))
                return items

            # ---------------- Phase 1: W + H branches per t-plane
            def load_x(p):
                xt = xtp.tile([128, 4, 1024], BF16, name="xt", tag="xt",
                              bufs=4)
                for cc in range(4):
                    nc.gpsimd.dma_start(
                        out=xt[:, cc, :],
                        in_=x_in[cc * 128 : (cc + 1) * 128, p, :, :]
                        .rearrange("p h w -> p (h w)"),
                    )
                return xt

            def load_x8(p):
                # fp8 copy of the plane in DoubleRow channel layout
                # (c = ic2*256 + ko*128 + partition), natural token order
                xt8 = xtp.tile([128, 2, 2, 1024], F8, name="xt8", tag="xt8",
                               bufs=3)
                for cc in range(4):
                    nc.gpsimd.dma_start(
                        out=xt8[:, cc // 2, cc % 2, :],
                        in_=x_in[cc * 128 : (cc + 1) * 128, p, :, :]
                        .rearrange("p h w -> p (h w)"),
                    )
                return xt8

            def make_xth(xt):
                # w-major reorder (GpSimd, which is otherwise idle),
                # pipelined one plane ahead so the copy's input DMA has
                # already landed when it issues (no head-of-line blocking)
                xth = xtp.tile([128, 4, 1024], BF16, name="xth", tag="xth",
                               bufs=3)
                nc.vector.tensor_copy(
                    out=xth.rearrange("p c (w h) -> p c w h", h=32),
                    in_=xt.rearrange("p c (h w) -> p c w h", w=32),
                )
                return xth

            # ---------------- Phase 2 helpers: T branch on ROW PAIRS
            # (two adjacent h-rows = 1024 tokens, same shape as a plane)
            def load_xn(j):
                r = 2 * j
                xn = xtp.tile([128, 4, 1024], BF16, name="xn", tag="xt",
                              bufs=4)
                for cc in range(4):
                    nc.gpsimd.dma_start(
                        out=xn[:, cc, :].rearrange(
                            "p (t r w) -> p t (r w)", r=2, w=32),
                        in_=x_in[cc * 128 : (cc + 1) * 128, :, r : r + 2, :]
                        .rearrange("p t r w -> p t (r w)"),
                    )
                return xn

            def make_xtt(xn):
                # per row: (w, t) reorder; rows stay in separate halves
                xtt = xtp.tile([128, 4, 1024], BF16, name="xtt", tag="xth",
                               bufs=3)
                nc.vector.tensor_copy(
                    out=xtt.rearrange("p c (r w t) -> p c r w t", r=2, w=32),
                    in_=xn.rearrange("p c (t r w) -> p c r w t", r=2, w=32),
                )
                return xtt

            def load_xn8(j):
                r = 2 * j
                xn8 = xtp.tile([128, 2, 2, 1024], F8, name="xn8", tag="xt8",
                               bufs=3)
                for cc in range(4):
                    nc.gpsimd.dma_start(
                        out=xn8[:, cc // 2, cc % 2, :].rearrange(
                            "p (t rw) -> p t rw", rw=64),
                        in_=x_in[cc * 128 : (cc + 1) * 128, :, r : r + 2, :]
                        .rearrange("p t r w -> p t (r w)"),
                    )
                return xn8

            # ---------------- Unit stream driver: every unit (W-plane,
            # H-plane, T row-pair) is [front: projections] [attn: scores +
            # softmax] [back: AV + out-projection]. The loop runs
            # attn(i-1), front(i), back(i-1) so each unit's softmax chains
            # drain underneath the next unit's ~10us of big matmuls.
            xts = {0: load_x(0), 1: load_x(1), 2: load_x(2)}
            xths = {0: make_xth(xts[0]), 1: make_xth(xts[1])}
            xt8s = {0: load_x8(0), 1: load_x8(1)}
            xns = {}
            xtts = {}
            xn8s = {}

            def drain(oc, tt, yps, ysb):
                # plain contiguous PSUM drain: every branch's out-proj chunk
                # tt lands at ysb offset tt*512 in its own token order
                dst = ysb[:, oc, tt * 512 : (tt + 1) * 512]
                if (oc + 2 * tt) % 2 == 0:
                    nc.scalar.copy(out=dst, in_=yps)
                else:
                    nc.vector.tensor_copy(out=dst, in_=yps)

            def units():
                for p in range(T):
                    if p + 3 < T:
                        xts[p + 3] = load_x(p + 3)
                    if p + 2 < T:
                        xths[p + 2] = make_xth(xts[p + 2])
                        xt8s[p + 2] = load_x8(p + 2)
                    if p == T - 2:
                        # phase-2 prologue: issue early so pair-0/1 inputs
                        # land while planes 14-15 compute
                        xns[0] = load_xn(0)
                    if p == T - 1:
                        xns[1] = load_xn(1)
                        xns[2] = load_xn(2)
                        xtts[0] = make_xtt(xns[0])
                        xtts[1] = make_xtt(xns[1])
                    xt = xts.pop(p)
                    xth = xths.pop(p)
                    xt8 = xt8s.pop(p)
                    ysbw = yp.tile([128, 4, 1024], BF16, name="ysbw",
                                   tag="ysb", bufs=3)
                    ysbh = yp.tile([128, 4, 1024], BF16, name="ysbh",
                                   tag="ysb", bufs=3)

                    def xv8(ic2, tt, xt8=xt8):
                        return xt8[:, ic2, :, tt * 512 : (tt + 1) * 512]

                    def xv_h(ic, tt, xth=xth):
                        return xth[:, ic, tt * 512 : (tt + 1) * 512]

                    def xg_w(ic, g, xt=xt):
                        return xt[:, ic, g * 128 : (g + 1) * 128]

                    def xg_h(ic, g, xth=xth):
                        return xth[:, ic, g * 128 : (g + 1) * 128]

                    def wr_w(oc, tt, yps, ysbw=ysbw):
                        drain(oc, tt, yps, ysbw)

                    def wr_h(oc, tt, yps, ysbh=ysbh):
                        drain(oc, tt, yps, ysbh)

                    def post_w(p=p, ysbw=ysbw):
                        for cc in range(4):
                            nc.sync.dma_start(
                                out=y_w[cc * 128 : (cc + 1) * 128, p, :],
                                in_=ysbw[:, cc, :],
                            )

                    def post_h(p=p, ysbh=ysbh):
                        for cc in range(4):
                            nc.sync.dma_start(
                                out=y_h[cc * 128 : (cc + 1) * 128, p, :],
                                in_=ysbh[:, cc, :],
                            )

                    yield ("w", xv8, xg_w, False, wr_w, post_w)
                    yield ("h", xv_h, xg_h, False, wr_h, post_h)

                for j in range(H // 2):
                    if j + 3 < H // 2:
                        xns[j + 3] = load_xn(j + 3)
                    if j + 2 < H // 2:
                        xtts[j + 2] = make_xtt(xns[j + 2])
                    xns.pop(j, None)
                    xtt = xtts.pop(j)
                    ysbt = yp.tile([128, 4, 1024], BF16, name="ysbt",
                                   tag="ysb", bufs=3)

                    def xv_t(ic, tt, xtt=xtt):
                        return xtt[:, ic, tt * 512 : (tt + 1) * 512]

                    def xg_t(ic, g, xtt=xtt):
                        return xtt[:, ic, g * 128 : (g + 1) * 128]

                    def wr2(oc, tt, yps, ysbt=ysbt):
                        drain(oc, tt, yps, ysbt)

                    def post_t(j=j, ysbt=ysbt):
                        r = 2 * j
                        for cc in range(4):
                            nc.sync.dma_start(
                                out=y_t[cc * 128 : (cc + 1) * 128,
                                        r : r + 2, :],
                                in_=ysbt[:, cc, :].rearrange(
                                    "p (r wt) -> p r wt", r=2),
                            )

                    yield ("t", xv_t, xg_t, True, wr2, post_t)

            pends = []
            for ax, xv, xg, masked, write_fn, post in units():
                fillers = (attn_fillers(pends[0][0], pends[0][1], pends[0][2])
                           if len(pends) == 2 else [])
                st = branch_front(ax, 1024, xv, xg, fillers)
                if len(pends) == 2:
                    done = pends.pop(0)
                    if done[3] is not None:
                        done[3]()
                pends.append((st, masked, write_fn, post))
            for st_, masked_, wf_, post_ in pends:
                for f in attn_fillers(st_, masked_, wf_):
                    f()
                if post_ is not None:
                    post_()
    nc.compile()
    return nc


_NC_CACHE = {}


def _get_nc():
    if "nc" not in _NC_CACHE:
        _NC_CACHE["nc"] = build_nc()
    return _NC_CACHE["nc"]


def kernel(x, wq_w, wk_w, wv_w, fc_w, fb_w,
           wq_h, wk_h, wv_h, fc_h, fb_h,
           wq_t, wk_t, wv_t, fc_t, fb_t, _trace=False):
    x = np.asarray(x, np.float32)
    scale = 1.0 / np.sqrt(np.float32(D))
    branches = {
        "w": (np.asarray(wq_w, np.float32), np.asarray(wk_w, np.float32),
              np.asarray(wv_w, np.float32), np.asarray(fc_w, np.float32)),
        "h": (np.asarray(wq_h, np.float32), np.asarray(wk_h, np.float32),
              np.asarray(wv_h, np.float32), np.asarray(fc_h, np.float32)),
        "t": (np.asarray(wq_t, np.float32), np.asarray(wk_t, np.float32),
              np.asarray(wv_t, np.float32), np.asarray(fc_t, np.float32)),
    }
    fb_sum = (np.asarray(fb_w, np.float32) + np.asarray(fb_h, np.float32)
              + np.asarray(fb_t, np.float32))

    import ml_dtypes

    in_maps = []
    for core in range(8):
        b, hh = core // 2, core % 2
        m = {"x_in": np.ascontiguousarray(x[b])}
        cols = slice(hh * CH, (hh + 1) * CH)
        for ax, (wq, wk, wvm, fcm) in branches.items():
            # q UNSCALED everywhere: 1/sqrt(d) is applied by exp's scale
            m[f"wqkv_{ax}"] = np.ascontiguousarray(
                np.concatenate(
                    [wq[:, cols], wk[:, cols], wvm[:, cols]], axis=1
                )
            )
            m[f"fc_{ax}"] = np.ascontiguousarray(fcm[cols, :])
        # W branch q/k additionally in fp8e4m3 for DoubleRow
        m["wqk8_w"] = np.ascontiguousarray(
            np.concatenate(
                [branches["w"][0][:, cols], branches["w"][1][:, cols]],
                axis=1,
            )
        ).astype(ml_dtypes.float8_e4m3)
        in_maps.append(m)

    nc = _get_nc()
    res = run_bass_kernel_spmd(
        nc, in_maps, core_ids=list(range(8)), trace=_trace,
    )
    y = np.empty((B, C, T, H, W), np.float32)
    for b in range(B):
        acc = None
        for core in (2 * b, 2 * b + 1):
            r = res.results[core]
            # y_w: [C, T, H*W] natural; y_h: [C, T, W*H] w-major;
            # y_t: [C, H, W*T] (h, w, t)-major
            part = r["y_w"].astype(np.float32).reshape(C, T, H, W)
            part = part + r["y_h"].astype(np.float32).reshape(C, T, W, H).transpose(0, 1, 3, 2)
            part = part + r["y_t"].astype(np.float32).reshape(C, H, W, T).transpose(0, 3, 1, 2)
            acc = part if acc is None else acc + part
        y[b] = acc
    y += fb_sum[None, :, None, None, None]
    if _trace:
        _NC_CACHE["last_result"] = res
    return y



# revision 68
# speedup vs baseline: 1.0098x; 1.0098x over previous
"""Trainium2 Bass kernel for nn_AxialBlock (axial attention, branches W/H/T).

Self-contained: accepts FULL inputs as in reference.setup_inputs(), shards
across 8 NeuronCores as (batch x head-half), runs one SPMD Bass program,
gathers on host.

Hardcoded problem shape: x (4, 512, 16, 32, 32) f32, C=512, 8 heads, d=64.

Per-core layout: activations channel-major [C, tokens]. The work is a single
stream of 48 uniform 1024-token "units": 16 t-planes x (W branch, H branch)
then 16 h-row-pairs (T branch, combined with the W+H partial from a DRAM
scratch). Branch token orders (W: (h,w) natural; H: (w,h); T: (r,w,t)) come
from DVE reorder copies pipelined one unit ahead.

The scheduling is the point: for consecutive units, the loop emits
  [unit i-1's attention/out-proj items interleaved inside unit i's
   projection acc-groups]
so the PE sees an unbroken dense matmul stream (stays at its fast p-state)
while each unit's softmax chain (Act exp -> DVE reduce/reciprocal ->
normalize muls split Act/DVE -> DVE transpose) drains underneath the next
unit's projections. Attention per 128-token group (4 sequences x 32 tokens)
is tile_position-packed; the two concurrently-streaming PE row groups write
separate PSUM banks (h strides one bank in the score tile) -- sharing a bank
between row groups faults on hardware.
"""

import numpy as np

import concourse.bass as bass
import concourse.mybir as mybir
from concourse import bacc, tile
from concourse.bass_utils import run_bass_kernel_spmd

F32 = mybir.dt.float32
F32R = mybir.dt.float32r
BF16 = mybir.dt.bfloat16
F8 = mybir.dt.float8e4
AF = mybir.ActivationFunctionType
ALU = mybir.AluOpType

B, C, T, H, W = 4, 512, 16, 32, 32
NH, D = 8, 64
HH = 4  # heads per core (head-half)
CH = HH * D  # 256 channels per core
NEG = -30000.0


def build_nc():
    nc = bacc.Bacc("TRN2", target_bir_lowering=False, debug=False, num_devices=8)

    x_in = nc.dram_tensor("x_in", [C, T, H, W], F32, kind="ExternalInput")
    # qkv weights (q UNSCALED -- the 1/sqrt(d) folds into exp's scale); the
    # W branch's q/k additionally pre-cast to fp8e4m3 for DoubleRow
    wqkv = {
        ax: nc.dram_tensor(f"wqkv_{ax}", [C, 3 * CH], F32, kind="ExternalInput")
        for ax in ("w", "h", "t")
    }
    fc = {
        ax: nc.dram_tensor(f"fc_{ax}", [CH, C], F32, kind="ExternalInput")
        for ax in ("w", "h", "t")
    }
    # One output per branch, each in that branch's natural token order
    # (contiguous drains + contiguous DMA); the host transposes and sums.
    y_w = nc.dram_tensor("y_w", [C, T, H * W], BF16, kind="ExternalOutput")
    y_h = nc.dram_tensor("y_h", [C, T, W * H], BF16, kind="ExternalOutput")
    y_t = nc.dram_tensor("y_t", [C, H, W * T], BF16, kind="ExternalOutput")

    # T-branch pair mask over (h, c, g2, m): within a 32-token col strip the
    # two 16-token sequences must not attend to each other.
    mrows = np.arange(128) % 32
    mcols = np.arange(32)
    m2 = np.where((mrows[:, None] // 16) == (mcols[None, :] // 16), 0.0, NEG)
    mask_np = np.broadcast_to(
        m2[:, None, None, None, :], (128, 2, 2, 4, 32)
    ).astype(np.float32)
    mask_dram = nc.inline_tensor(np.ascontiguousarray(mask_np), name="tmask")

    with tile.TileContext(nc) as tc:
        with (
            tc.tile_pool(name="consts", bufs=1) as consts,
            tc.tile_pool(name="xtp", bufs=2) as xtp,
            tc.tile_pool(name="qkv", bufs=2) as qkvp,
            tc.tile_pool(name="att", bufs=4) as attp,
            tc.tile_pool(name="yp", bufs=2) as yp,
            tc.tile_pool(name="ps", bufs=1, space="PSUM") as ps,
        ):
            w_t = {}
            fc_t = {}
            for ax in ("w", "h", "t"):
                w_t[ax] = consts.tile([128, 4, 3 * CH], BF16, name=f"w_{ax}")
                nc.gpsimd.dma_start(
                    out=w_t[ax],
                    in_=wqkv[ax].rearrange("(kc kp) m -> kp kc m", kp=128),
                )
                fc_t[ax] = consts.tile([128, 2, C], F32R, name=f"fc_{ax}")
                nc.sync.dma_start(
                    out=fc_t[ax],
                    in_=fc[ax].rearrange("(kc kp) m -> kp kc m", kp=128)
                    .bitcast(F32R),
                )
            mask_t = consts.tile([128, 2, 2, 4, 32], F32, name="mask_t")
            nc.sync.dma_start(out=mask_t, in_=mask_dram[:, :, :, :, :])

            def qk_mms(xv, ax, ntok, qt, kt, fill):
                """q/k projection. For the W branch (whose branch token order
                IS the natural plane order) the matmuls run in fp8 DoubleRow:
                2 matmuls per acc group over 256 contraction channels each,
                with xv(ic2, tt) -> [128, 2, 512] fp8 rhs. H/T branches run
                bf16 (4 matmuls) on their reordered x tiles -- an fp8 copy in
                branch order would cost more to produce than DoubleRow saves.
                Drains are contiguous either way. fill() runs one interleaved
                attention item per acc-group."""
                for tt in range(ntok // 512):
                    for mc in range(4):  # q0 q1 k0 k1
                        dst = qt if mc < 2 else kt
                        oc = mc % 2
                        acc = ps.tile([128, 512], F32, name="acc", tag="big",
                                      bufs=2)
                        for ic in range(4):
                            nc.tensor.matmul(
                                acc,
                                w_t[ax][:, ic, mc * 128 : (mc + 1) * 128],
                                xv(ic, tt),
                                start=(ic == 0),
                                stop=(ic == 3),
                            )
                        nc.scalar.copy(
                            out=dst[:, oc, tt * 512 : (tt + 1) * 512],
                            in_=acc,
                        )
                        fill()

            def v_mms(xg, ax, ntok, vr, fill):
                """v projection, token-major. xg: callable(ic, g) -> lhsT AP
                [128, 128] = tokens g*128..+128 in branch order. Two fill
                points per group (one mid-accumulation) so the filler list
                drains before the unit boundary -- trailing drains otherwise
                stall the next unit's first acc groups on the PSUM WAR."""
                for gp in range(0, ntok // 128, 2):
                    # two groups share one acc tile (1 bank): full-128-row
                    # stationaries never co-stream, so same-bank is safe;
                    # ONE drain covers both groups
                    acc2 = ps.tile([128, 2, 256], F32, name="acc2",
                                   tag="big", bufs=2)
                    for gj in range(2):
                        for ic in range(4):
                            nc.tensor.matmul(
                                acc2[:, gj, :],
                                xg(ic, gp + gj),
                                w_t[ax][:, ic, 512:768],
                                start=(ic == 0),
                                stop=(ic == 3),
                            )
                            if ic == 1:
                                fill()
                        fill()
                    if gp % 4 == 0:
                        nc.vector.tensor_copy(
                            out=vr[:, gp : gp + 2, :], in_=acc2)
                    else:
                        nc.scalar.copy(out=vr[:, gp : gp + 2, :], in_=acc2)

            def score_mms(g, sct, qt, kt):
                # sct [128, 2(h), 2(c), 8(g), 32(m)]: h strides one full PSUM
                # bank, so the two concurrently-streaming tile_position row
                # groups never share a bank (same constraint the per-h tiles
                # of the baseline satisfied).
                for s in range(4):
                    q32 = slice(g * 128 + s * 32, g * 128 + (s + 1) * 32)
                    for c in range(2):
                        for h in range(2):
                            nc.tensor.matmul(
                                sct[s * 32 : (s + 1) * 32, h, c, g, :],
                                qt[h * 64 : (h + 1) * 64, c, q32],
                                kt[h * 64 : (h + 1) * 64, c, q32],
                                start=True,
                                stop=True,
                                tile_position=(h * 64, s * 32),
                                skip_group_check=True,
                            )
            def softmax_quad(g, sct, masked):
                """Softmax for groups g..g+3 batched: one mask add, one exp,
                one reduce, one reciprocal, ONE broadcast normalize mul (on
                the otherwise-idle GpSimd engine; all operands SBUF), and ONE
                32x32-block stream transpose for the whole quad. The (h, c)
                layout is kept end-to-end; av_mms indexes attT accordingly.
                Returns the quad's attT tile."""
                scv = sct[:, :, :, g : g + 4, :].rearrange(
                    "p h c g m -> p h c (g m)"
                )
                if masked:
                    nc.vector.tensor_tensor(
                        out=scv, in0=scv,
                        in1=mask_t.rearrange("p h c g m -> p h c (g m)"),
                        op=ALU.add,
                    )
                # e4 memory layout (h, c, g4, m) == the exp input order
                e4 = attp.tile([128, 2, 2, 4, 32], BF16, name="e4",
                               tag="e2", bufs=4)
                # scores are unscaled (fp8 weights can't carry the tiny
                # pre-scaled q); 1/sqrt(d) folds into the exp input scale
                nc.scalar.activation(
                    out=e4.rearrange("p h c g m -> p h c (g m)"),
                    in_=scv,
                    func=AF.Exp,
                    scale=0.125,
                )
                rs = attp.tile([128, 16], F32, name="rs", tag="rs", bufs=4)
                nc.vector.tensor_reduce(
                    out=rs,
                    in_=e4.rearrange("p h c g m -> p (h c g) m"),
                    axis=mybir.AxisListType.X,
                    op=ALU.add,
                )
                rv = attp.tile([128, 16], F32, name="rv", tag="rv", bufs=4)
                nc.vector.reciprocal(out=rv, in_=rs)
                attn = attp.tile([128, 2, 2, 4, 32], BF16, name="attn",
                                 tag="attn", bufs=4)
                nc.gpsimd.tensor_tensor(
                    out=attn.rearrange("p h c g m -> p (h c g) m"),
                    in0=e4.rearrange("p h c g m -> p (h c g) m"),
                    in1=rv[:, :, None].to_broadcast([128, 16, 32]),
                    op=ALU.mult,
                )
                attT = attp.tile([128, 2, 2, 4, 32], BF16, name="attT",
                                 tag="attT", bufs=4)
                nc.vector.transpose(
                    out=attT.rearrange("p h c g m -> p (h c g m)"),
                    in_=attn.rearrange("p h c g m -> p (h c g m)"),
                )
                return attT

            def av_mms(g, vr, attT, gj, ot):
                # avt [128, 4(s), 512]: every s row group gets its own PSUM
                # bank (strictly more separation than the old av0/av1
                # alternation), then ONE strided DVE drain for the group.
                avt = ps.tile([128, 4, 512], F32, name="av", tag="av",
                              bufs=1)
                for s in range(4):
                    for c in range(2):
                        for h in range(2):
                            nc.tensor.matmul(
                                avt[h * 64 : (h + 1) * 64, s,
                                    c * 32 : (c + 1) * 32],
                                vr[s * 32 : (s + 1) * 32, g,
                                   (2 * c + h) * 64 : (2 * c + h + 1) * 64],
                                attT[s * 32 : (s + 1) * 32, h, c, gj, :],
                                start=True,
                                stop=True,
                                tile_position=(s * 32, h * 64),
                                skip_group_check=True,
                            )
                # split the drain across both engines (halves run in
                # parallel and the bufs=1 WAR clears sooner)
                dst = ot[:, :, g * 128 : (g + 1) * 128].rearrange(
                    "p c (s m) -> p c s m", m=32
                )
                src = avt[:, :, 0:64].rearrange(
                    "p s (c m) -> p c s m", m=32
                )
                nc.vector.tensor_copy(out=dst[:, :, 0:2, :],
                                      in_=src[:, :, 0:2, :])
                nc.scalar.copy(out=dst[:, :, 2:4, :], in_=src[:, :, 2:4, :])

            def op_item(ax, ot, tt, oc, write_fn):
                yps = ps.tile([128, 512], F32, name="yps", tag="big",
                              bufs=2)
                for ic in range(2):
                    nc.tensor.matmul(
                        yps,
                        fc_t[ax][:, ic, oc * 128 : (oc + 1) * 128],
                        ot[:, ic, tt * 512 : (tt + 1) * 512],
                        start=(ic == 0),
                        stop=(ic == 1),
                    )
                write_fn(oc, tt, yps)

            def branch_front(ax, ntok, xv, xg, fillers):
                """Projections for one branch, with the attention items of
                the unit two slots back interleaved between acc-groups so the
                PE never sees a sparse stretch."""
                ng = ntok // 128
                qt = qkvp.tile([128, 2, 1024], BF16, name="qt", tag="qt",
                               bufs=3)[:, :, :ntok]
                kt = qkvp.tile([128, 2, 1024], BF16, name="kt", tag="kt",
                               bufs=3)[:, :, :ntok]
                vr = qkvp.tile([128, 8, 256], BF16, name="vr", tag="vr",
                               bufs=3)[:, :ng, :]
                ot = qkvp.tile([128, 2, 1024], F32R, name="ot", tag="ot",
                               bufs=2)[:, :, :ntok]
                it = iter(fillers)
                state = {"skip": 0}

                def fill():
                    if state["skip"] > 0:  # let qt/kt copies get ahead
                        state["skip"] -= 1
                        return
                    f = next(it, None)
                    if f is not None:
                        f()

                qk_mms(xv, ax, ntok, qt, kt, fill)
                v_mms(xg, ax, ntok, vr, fill)
                for f in it:
                    f()
                return (ax, ntok, qt, kt, vr, ot)

            def attn_fillers(st, masked, write_fn):
                """Interleavable attention + out-projection items for a unit:
                score packs, AV packs, and (tt, oc) out-proj chunks ordered so
                each item's dependencies were issued several items earlier."""
                ax, ntok, qt, kt, vr, ot = st
                ng = ntok // 128
                sct = ps.tile([128, 2, 2, 8, 32], F32, name="sc", tag="sc",
                              bufs=1)
                attTs = [None] * (ng // 4)
                items = []

                def sc_item(gq):
                    for g in range(gq, gq + 4):
                        score_mms(g, sct, qt, kt)
                    attTs[gq // 4] = softmax_quad(gq, sct, masked)

                def av_item(g):
                    av_mms(g, vr, attTs[g // 4], g % 4, ot)

                for gq in range(0, ng, 4):
                    items.append(lambda gq=gq: sc_item(gq))
                for g in range(ng // 2):
                    items.append(lambda g=g: av_item(g))
                for k in range(ng // 2):
                    items.append(lambda g=ng // 2 + k: av_item(g))
                    items.append(
                        lambda oc=k: op_item(ax, ot, 0, oc, write_fn))
                for oc in range(4):
                    items.append(lambda oc=oc: op_item(ax, ot, 1, oc,
                                                       write_fn))
                return items

            # ---------------- Phase 1: W + H branches per t-plane
            def load_x(p):
                xt = xtp.tile([128, 4, 1024], BF16, name="xt", tag="xt",
                              bufs=4)
                for cc in range(4):
                    nc.gpsimd.dma_start(
                        out=xt[:, cc, :],
                        in_=x_in[cc * 128 : (cc + 1) * 128, p, :, :]
                        .rearrange("p h w -> p (h w)"),
                    )
                return xt


            def make_xth(xt):
                # w-major reorder (GpSimd, which is otherwise idle),
                # pipelined one plane ahead so the copy's input DMA has
                # already landed when it issues (no head-of-line blocking)
                xth = xtp.tile([128, 4, 1024], BF16, name="xth", tag="xth",
                               bufs=3)
                nc.vector.tensor_copy(
                    out=xth.rearrange("p c (w h) -> p c w h", h=32),
                    in_=xt.rearrange("p c (h w) -> p c w h", w=32),
                )
                return xth

            # ---------------- Phase 2 helpers: T branch on ROW PAIRS
            # (two adjacent h-rows = 1024 tokens, same shape as a plane)
            def load_xn(j):
                r = 2 * j
                xn = xtp.tile([128, 4, 1024], BF16, name="xn", tag="xt",
                              bufs=4)
                for cc in range(4):
                    nc.gpsimd.dma_start(
                        out=xn[:, cc, :].rearrange(
                            "p (t r w) -> p t (r w)", r=2, w=32),
                        in_=x_in[cc * 128 : (cc + 1) * 128, :, r : r + 2, :]
                        .rearrange("p t r w -> p t (r w)"),
                    )
                return xn

            def make_xtt(xn):
                # per row: (w, t) reorder; rows stay in separate halves
                xtt = xtp.tile([128, 4, 1024], BF16, name="xtt", tag="xth",
                               bufs=3)
                nc.vector.tensor_copy(
                    out=xtt.rearrange("p c (r w t) -> p c r w t", r=2, w=32),
                    in_=xn.rearrange("p c (t r w) -> p c r w t", r=2, w=32),
                )
                return xtt


            # ---------------- Unit stream driver: every unit (W-plane,
            # H-plane, T row-pair) is [front: projections] [attn: scores +
            # softmax] [back: AV + out-projection]. The loop runs
            # attn(i-1), front(i), back(i-1) so each unit's softmax chains
            # drain underneath the next unit's ~10us of big matmuls.
            xts = {0: load_x(0), 1: load_x(1), 2: load_x(2)}
            xths = {0: make_xth(xts[0]), 1: make_xth(xts[1])}
            xns = {}
            xtts = {}

            def drain(oc, tt, yps, ysb):
                # plain contiguous PSUM drain: every branch's out-proj chunk
                # tt lands at ysb offset tt*512 in its own token order
                dst = ysb[:, oc, tt * 512 : (tt + 1) * 512]
                if (oc + 2 * tt) % 2 == 0:
                    nc.scalar.copy(out=dst, in_=yps)
                else:
                    nc.vector.tensor_copy(out=dst, in_=yps)

            def units():
                for p in range(T):
                    if p + 3 < T:
                        xts[p + 3] = load_x(p + 3)
                    if p + 2 < T:
                        xths[p + 2] = make_xth(xts[p + 2])
                    if p == T - 2:
                        # phase-2 prologue: issue early so pair-0/1 inputs
                        # land while planes 14-15 compute
                        xns[0] = load_xn(0)
                    if p == T - 1:
                        xns[1] = load_xn(1)
                        xns[2] = load_xn(2)
                        xtts[0] = make_xtt(xns[0])
                        xtts[1] = make_xtt(xns[1])
                    xt = xts.pop(p)
                    xth = xths.pop(p)
                    ysbw = yp.tile([128, 4, 1024], BF16, name="ysbw",
                                   tag="ysb", bufs=3)
                    ysbh = yp.tile([128, 4, 1024], BF16, name="ysbh",
                                   tag="ysb", bufs=3)

                    def xv_w(ic, tt, xt=xt):
                        return xt[:, ic, tt * 512 : (tt + 1) * 512]

                    def xv_h(ic, tt, xth=xth):
                        return xth[:, ic, tt * 512 : (tt + 1) * 512]

                    def xg_w(ic, g, xt=xt):
                        return xt[:, ic, g * 128 : (g + 1) * 128]

                    def xg_h(ic, g, xth=xth):
                        return xth[:, ic, g * 128 : (g + 1) * 128]

                    def wr_w(oc, tt, yps, ysbw=ysbw):
                        drain(oc, tt, yps, ysbw)

                    def wr_h(oc, tt, yps, ysbh=ysbh):
                        drain(oc, tt, yps, ysbh)

                    def post_w(p=p, ysbw=ysbw):
                        for cc in range(4):
                            nc.sync.dma_start(
                                out=y_w[cc * 128 : (cc + 1) * 128, p, :],
                                in_=ysbw[:, cc, :],
                            )

                    def post_h(p=p, ysbh=ysbh):
                        for cc in range(4):
                            nc.sync.dma_start(
                                out=y_h[cc * 128 : (cc + 1) * 128, p, :],
                                in_=ysbh[:, cc, :],
                            )

                    yield ("w", xv_w, xg_w, False, wr_w, post_w)
                    yield ("h", xv_h, xg_h, False, wr_h, post_h)

                for j in range(H // 2):
                    if j + 3 < H // 2:
                        xns[j + 3] = load_xn(j + 3)
                    if j + 2 < H // 2:
                        xtts[j + 2] = make_xtt(xns[j + 2])
                    xns.pop(j, None)
                    xtt = xtts.pop(j)
                    ysbt = yp.tile([128, 4, 1024], BF16, name="ysbt",
                                   tag="ysb", bufs=3)

                    def xv_t(ic, tt, xtt=xtt):
                        return xtt[:, ic, tt * 512 : (tt + 1) * 512]

                    def xg_t(ic, g, xtt=xtt):
                        return xtt[:, ic, g * 128 : (g + 1) * 128]

                    def wr2(oc, tt, yps, ysbt=ysbt):
                        drain(oc, tt, yps, ysbt)

                    def post_t(j=j, ysbt=ysbt):
                        r = 2 * j
                        for cc in range(4):
                            nc.sync.dma_start(
                                out=y_t[cc * 128 : (cc + 1) * 128,
                                        r : r + 2, :],
                                in_=ysbt[:, cc, :].rearrange(
                                    "p (r wt) -> p r wt", r=2),
                            )

                    yield ("t", xv_t, xg_t, True, wr2, post_t)

            pends = []
            for ax, xv, xg, masked, write_fn, post in units():
                fillers = (attn_fillers(pends[0][0], pends[0][1], pends[0][2])
                           if len(pends) == 2 else [])
                st = branch_front(ax, 1024, xv, xg, fillers)
                if len(pends) == 2:
                    done = pends.pop(0)
                    if done[3] is not None:
                        done[3]()
                pends.append((st, masked, write_fn, post))
            for st_, masked_, wf_, post_ in pends:
                for f in attn_fillers(st_, masked_, wf_):
                    f()
                if post_ is not None:
                    post_()
    nc.compile()
    return nc


_NC_CACHE = {}


def _get_nc():
    if "nc" not in _NC_CACHE:
        _NC_CACHE["nc"] = build_nc()
    return _NC_CACHE["nc"]


def kernel(x, wq_w, wk_w, wv_w, fc_w, fb_w,
           wq_h, wk_h, wv_h, fc_h, fb_h,
           wq_t, wk_t, wv_t, fc_t, fb_t, _trace=False):
    x = np.asarray(x, np.float32)
    scale = 1.0 / np.sqrt(np.float32(D))
    branches = {
        "w": (np.asarray(wq_w, np.float32), np.asarray(wk_w, np.float32),
              np.asarray(wv_w, np.float32), np.asarray(fc_w, np.float32)),
        "h": (np.asarray(wq_h, np.float32), np.asarray(wk_h, np.float32),
              np.asarray(wv_h, np.float32), np.asarray(fc_h, np.float32)),
        "t": (np.asarray(wq_t, np.float32), np.asarray(wk_t, np.float32),
              np.asarray(wv_t, np.float32), np.asarray(fc_t, np.float32)),
    }
    fb_sum = (np.asarray(fb_w, np.float32) + np.asarray(fb_h, np.float32)
              + np.asarray(fb_t, np.float32))

    in_maps = []
    for core in range(8):
        b, hh = core // 2, core % 2
        m = {"x_in": np.ascontiguousarray(x[b])}
        cols = slice(hh * CH, (hh + 1) * CH)
        for ax, (wq, wk, wvm, fcm) in branches.items():
            # q UNSCALED everywhere: 1/sqrt(d) is applied by exp's scale
            m[f"wqkv_{ax}"] = np.ascontiguousarray(
                np.concatenate(
                    [wq[:, cols], wk[:, cols], wvm[:, cols]], axis=1
                )
            )
            m[f"fc_{ax}"] = np.ascontiguousarray(fcm[cols, :])
        in_maps.append(m)

    nc = _get_nc()
    res = run_bass_kernel_spmd(
        nc, in_maps, core_ids=list(range(8)), trace=_trace,
    )
    y = np.empty((B, C, T, H, W), np.float32)
    for b in range(B):
        acc = None
        for core in (2 * b, 2 * b + 1):
            r = res.results[core]
            # y_w: [C, T, H*W] natural; y_h: [C, T, W*H] w-major;
            # y_t: [C, H, W*T] (h, w, t)-major
            part = r["y_w"].astype(np.float32).reshape(C, T, H, W)
            part = part + r["y_h"].astype(np.float32).reshape(C, T, W, H).transpose(0, 1, 3, 2)
            part = part + r["y_t"].astype(np.float32).reshape(C, H, W, T).transpose(0, 3, 1, 2)
            acc = part if acc is None else acc + part
        y[b] = acc
    y += fb_sum[None, :, None, None, None]
    if _trace:
        _NC_CACHE["last_result"] = res
    return y

